# revision 11
# baseline (speedup 1.0000x reference)
"""Learnable 3D Gaussian field evaluation on 8 Trainium2 NeuronCores.

Reference computes, for B=32768 points x and N=4096 gaussians
(mean m_n, packed Cholesky cov_tril, weight w_n):

    out[b] = sum_n w_n * exp(-0.5 * (x_b - m_n)^T A_n (x_b - m_n)),
    A_n = (L_n L_n^T)^{-1}

Two key reformulations:

1. Quadratic-form matmul: the exponent is a quadratic in x, so with a
   10-dim feature vector f(x) = [x0^2, x1^2, x2^2, x0x1, x0x2, x1x2,
   x0, x1, x2, 1] (x centered) and per-gaussian coefficients c_n (with
   -0.5 and log w_n folded in), exponent[b, n] = f(x_b) . c_n — a
   TensorEngine matmul followed by exp + row-sum on the ScalarEngine
   (one activation instruction with accum_out). fp32 operands are each
   split into 3 bf16 components and the 6 significant cross products
   stacked along the contraction dim (K=60) -> ~fp32 precision.

2. Certified spatial culling: the exp work (B*N/8 elements per core on
   the only engine with an exp LUT) is the roofline. Points are
   Morton-sorted into 256 tiles of 128; for each tile, gaussians whose
   maximum possible contribution (upper bound via distance to the tile
   bbox and the largest covariance eigenvalue) is negligible are
   dropped. The drop budget is adaptive: sum of dropped upper bounds
   <= 1e-3 * (certified lower bound of the output anywhere in the
   tile), so the relative error is bounded by ~1e-3 per point by
   construction, for any input. Survivors (~12-15% here) are gathered
   on the host into dense per-tile coefficient slabs.

SPMD scheduling: all 8 cores share one instruction stream, so tile
work units (split at 2048 columns) are sorted by width and dealt
round-robin into groups of 8 — one slot per group, slot width = group
max. Per-core work is identical and balanced; each core's packed
operands carry its own unit's features/coefficients. Padding columns
encode exponent -30000 so they contribute exp(-30000) = 0.

Sharding: slots are data-parallel across 8 cores; no collectives.
"""

import sys

import numpy as np

try:
    import concourse.bass as bass  # noqa: F401
except ImportError:
    sys.path.insert(0, "/opt/trn_rl_repo")

import ml_dtypes

import concourse.bacc as bacc
import concourse.bass as bass  # noqa: F401
import concourse.mybir as mybir
import concourse.tile as tile
from concourse.bass_utils import run_bass_kernel_spmd

B, N = 32768, 4096
N_CORES = 8
TILE_PTS = 128                  # points per tile (PSUM partition dim)
N_TILES = B // TILE_PTS         # 256 spatial point-tiles
MAX_W = 2048                    # max slot width (half of PSUM, 4 banks)
KSPLIT = 60                     # 6 bf16 cross products x 10 features
CENTER = 5.0
CULL_REL = 1e-3                 # culling error budget vs per-tile lower bound
PAD_EXP = -30000.0              # exponent encoded by padding columns

BF16 = mybir.dt.bfloat16
F32 = mybir.dt.float32


# ---------------------------------------------------------------- host math

def _build_coeffs(means, cov_tril, weights):
    """[N, 10] float64 coefficients c_n so that exponent = f(x') . c_n."""
    m = means.astype(np.float64) - CENTER
    ct = cov_tril.astype(np.float64)
    w = weights.astype(np.float64)
    eps = 1e-6
    L00 = np.exp(ct[:, 0]) + eps
    L11 = np.exp(ct[:, 2]) + eps
    L22 = np.exp(ct[:, 5]) + eps
    L10, L20, L21 = ct[:, 1], ct[:, 3], ct[:, 4]
    i00 = 1.0 / L00
    i11 = 1.0 / L11
    i22 = 1.0 / L22
    i10 = -L10 / (L00 * L11)
    i21 = -L21 / (L11 * L22)
    i20 = (L10 * L21 - L20 * L11) / (L00 * L11 * L22)
    A00 = i00 * i00 + i10 * i10 + i20 * i20
    A01 = i10 * i11 + i20 * i21
    A02 = i20 * i22
    A11 = i11 * i11 + i21 * i21
    A12 = i21 * i22
    A22 = i22 * i22
    Am0 = A00 * m[:, 0] + A01 * m[:, 1] + A02 * m[:, 2]
    Am1 = A01 * m[:, 0] + A11 * m[:, 1] + A12 * m[:, 2]
    Am2 = A02 * m[:, 0] + A12 * m[:, 1] + A22 * m[:, 2]
    mAm = m[:, 0] * Am0 + m[:, 1] * Am1 + m[:, 2] * Am2
    return np.stack(
        [
            -0.5 * A00, -0.5 * A11, -0.5 * A22,
            -A01, -A02, -A12,
            Am0, Am1, Am2,
            -0.5 * mAm + np.log(w),
        ],
        axis=1,
    )


def _build_feats(x):
    """[B, 10] float64 features of centered x."""
    xc = x.astype(np.float64) - CENTER
    x0, x1, x2 = xc[:, 0], xc[:, 1], xc[:, 2]
    return np.stack(
        [x0 * x0, x1 * x1, x2 * x2, x0 * x1, x0 * x2, x1 * x2,
         x0, x1, x2, np.ones_like(x0)],
        axis=1,
    )


def _split3_bf16(a64):
    """float64 -> three bf16 components with p0+p1+p2 covering ~24 bits."""
    p0 = a64.astype(ml_dtypes.bfloat16)
    r1 = a64 - p0.astype(np.float64)
    p1 = r1.astype(ml_dtypes.bfloat16)
    r2 = r1 - p1.astype(np.float64)
    p2 = r2.astype(ml_dtypes.bfloat16)
    return p0, p1, p2


def _split_stacks(f, c):
    """[60, B] and [60, N] bf16 stacks of the 6 significant cross products."""
    F0, F1, F2 = _split3_bf16(f)
    C0, C1, C2 = _split3_bf16(c)
    pairs = [(F0, C0), (F0, C1), (F1, C0), (F0, C2), (F1, C1), (F2, C0)]
    fstack = np.ascontiguousarray(
        np.concatenate([p[0].T for p in pairs], axis=0)
    ).astype(ml_dtypes.bfloat16)
    cstack = np.ascontiguousarray(
        np.concatenate([p[1].T for p in pairs], axis=0)
    ).astype(ml_dtypes.bfloat16)
    return fstack, cstack


def _kd_order(x):
    """Recursive median split (longest axis) into tiles of TILE_PTS points:
    compact bounding boxes everywhere, which is what the culling bound
    feeds on. Split sizes are kept multiples of TILE_PTS so every leaf is
    exactly one tile."""
    out = []

    def rec(ids):
        if len(ids) <= TILE_PTS:
            out.append(ids)
            return
        p = x[ids]
        ax = int(np.argmax(p.max(0) - p.min(0)))
        half = max(TILE_PTS, (len(ids) // 2 // TILE_PTS) * TILE_PTS)
        part = np.argpartition(p[:, ax], half)
        rec(ids[part[:half]])
        rec(ids[part[half:]])

    rec(np.arange(len(x)))
    return np.concatenate(out)


def _sigma_bounds(cov_tril):
    """Per-gaussian sqrt of smallest/largest eigenvalue of L L^T."""
    ct = cov_tril.astype(np.float64)
    eps = 1e-6
    L = np.zeros((N, 3, 3))
    L[:, 0, 0] = np.exp(ct[:, 0]) + eps
    L[:, 1, 1] = np.exp(ct[:, 2]) + eps
    L[:, 2, 2] = np.exp(ct[:, 5]) + eps
    L[:, 1, 0] = ct[:, 1]
    L[:, 2, 0] = ct[:, 3]
    L[:, 2, 1] = ct[:, 4]
    ev = np.linalg.eigvalsh(L @ L.transpose(0, 2, 1))
    return np.sqrt(np.maximum(ev[:, 0], 1e-30)), np.sqrt(ev[:, -1])


def _cull_tiles(xs, means, sig_min, sig_max, absw):
    """Per-tile certified culling of a tiled point set [T*TILE_PTS, 3].
    Drops gaussians with the smallest contribution upper bounds until the
    dropped total reaches CULL_REL times the certified output lower bound
    anywhere in the tile. Returns survivor index arrays per tile."""
    tiles = xs.reshape(-1, TILE_PTS, 3)
    lo = tiles.min(axis=1)
    hi = tiles.max(axis=1)
    surv = []
    for t in range(len(tiles)):
        dv = np.maximum(np.maximum(lo[t][None, :] - means,
                                   means - hi[t][None, :]), 0)
        dnear2 = (dv ** 2).sum(1)
        fv = np.maximum(np.abs(means - lo[t][None, :]),
                        np.abs(means - hi[t][None, :]))
        dfar2 = (fv ** 2).sum(1)
        ub = absw * np.exp(-0.5 * dnear2 / sig_max ** 2)
        out_lb = (absw * np.exp(-0.5 * dfar2 / sig_min ** 2)).sum()
        o = np.argsort(ub)
        ndrop = int(np.searchsorted(np.cumsum(ub[o]), CULL_REL * out_lb))
        surv.append(np.sort(o[ndrop:]).astype(np.int64))
    return surv


def _prepare(inputs):
    """Host prep: sort, cull, pack. Returns (in_maps, slot_widths, units,
    order) where units[s][c] = (tile, survivor_cols) for slot s, core c."""
    x = inputs["x"].astype(np.float64)
    means = inputs["means"].astype(np.float64)
    cov_tril = inputs["cov_tril"]
    weights = inputs["weights"].astype(np.float64)

    sig_min, sig_max = _sigma_bounds(cov_tril)
    absw = np.maximum(np.abs(weights), 1e-30)

    # pass 1: spatial sort + certified culling; find tiles blown up by
    # low-density outlier points (tiny certified lower bound -> huge
    # survivor count)
    order = _kd_order(x)
    surv = _cull_tiles(x[order], means, sig_min, sig_max, absw)
    fat = [t for t in range(len(surv)) if len(surv[t]) > 1024]
    cand = (np.concatenate([order[t * TILE_PTS:(t + 1) * TILE_PTS]
                            for t in fat])
            if fat else np.empty(0, np.int64))
    if len(cand) >= TILE_PTS:
        # exact per-point lower bound for candidate points; the TILE_PTS
        # worst become one dedicated tile so they stop inflating the
        # culling budget of whole spatial tiles
        d2 = ((x[cand][:, None, :] - means[None, :, :]) ** 2).sum(-1)
        lb_pt = (absw[None, :]
                 * np.exp(-0.5 * d2 / sig_min[None, :] ** 2)).sum(1)
        out_ids = cand[np.argsort(lb_pt)[:TILE_PTS]]
        rest = np.setdiff1d(np.arange(B), out_ids)
        order = np.concatenate([rest[_kd_order(x[rest])], out_ids])
        surv = _cull_tiles(x[order], means, sig_min, sig_max, absw)
    xs = x[order]

    # work units (tile, survivor column slice), split at MAX_W
    raw_units = []
    for t in range(N_TILES):
        cols = surv[t]
        for off in range(0, len(cols), MAX_W):
            raw_units.append((t, cols[off:off + MAX_W]))
    raw_units.sort(key=lambda u: -len(u[1]))
    n_slots = (len(raw_units) + N_CORES - 1) // N_CORES
    units = []          # units[s][c] -> (tile, cols) or None
    slot_widths = []
    for s in range(n_slots):
        grp = raw_units[s * N_CORES:(s + 1) * N_CORES]
        w = max(len(u[1]) for u in grp)
        w = ((w + 127) // 128) * 128
        slot_widths.append(w)
        grp = grp + [None] * (N_CORES - len(grp))
        units.append(grp)

    # packed per-core operands
    feats64 = _build_feats(xs)
    coeffs64 = _build_coeffs(means, cov_tril, weights)
    fstack, cstack = _split_stacks(feats64, coeffs64)      # [60,B], [60,N]
    totc = int(np.sum(slot_widths))
    pad_col = np.zeros((64,), dtype=ml_dtypes.bfloat16)
    pad_col[9] = PAD_EXP       # block-0 constant-feature row -> exp -> 0

    in_maps = []
    for c in range(N_CORES):
        ft = np.zeros((64, n_slots * TILE_PTS), dtype=ml_dtypes.bfloat16)
        cf = np.tile(pad_col[:, None], (1, totc))
        coff = 0
        for s in range(n_slots):
            u = units[s][c]
            if u is not None:
                t, cols = u
                ft[:KSPLIT, s * TILE_PTS:(s + 1) * TILE_PTS] = \
                    fstack[:, t * TILE_PTS:(t + 1) * TILE_PTS]
                cf[:KSPLIT, coff:coff + len(cols)] = cstack[:, cols]
                cf[KSPLIT:, coff:coff + len(cols)] = 0.0
            coff += slot_widths[s]
        in_maps.append({"feats": np.ascontiguousarray(ft),
                        "coeffs": np.ascontiguousarray(cf)})
    return in_maps, slot_widths, units, order


# ------------------------------------------------------------- device kernel

def _dedup_ldweights(nc):
    """Remove redundant InstLdweights: consecutive matmuls reusing the same
    stationary operand only need the first load. Only drops loads that carry
    no semaphore waits/updates and whose weights AP matches the previous
    load, with nothing but matmuls in between on the PE stream."""
    removed = 0
    for blk in nc.m.functions[0].blocks:
        keep = []
        last_sig = None
        for ins in blk.instructions:
            if getattr(ins, "engine", None) == mybir.EngineType.PE:
                tname = type(ins).__name__
                if tname == "InstLdweights":
                    sig = repr(ins.ins[0])
                    if sig == last_sig and ins.sync_info is None:
                        removed += 1
                        continue
                    last_sig = sig
                elif tname != "InstMatmult":
                    last_sig = None
            keep.append(ins)
        if removed:
            del blk.instructions[:]
            for ins in keep:
                blk.instructions.append(ins)
    return removed


_ENGINE_SEM_PREFIX = {
    mybir.EngineType.PE: "PE_",
    mybir.EngineType.Activation: "Activation_",
}


def _strip_self_waits(nc):
    """Drop same-engine semaphore waits from multi-wait PE/ACT instructions.

    Engines execute their instruction streams in order, so a wait on the
    engine's own completion semaphore is redundant whenever the instruction
    also carries the cross-engine wait that actually orders it."""
    n = 0
    for blk in nc.m.functions[0].blocks:
        for ins in blk.instructions:
            pfx = _ENGINE_SEM_PREFIX.get(getattr(ins, "engine", None))
            si = ins.sync_info
            if pfx is None or si is None or not si.on_wait:
                continue
            waits = list(si.on_wait)
            if len(waits) < 2:
                continue
            kept = [w for w in waits if not w.ant_name.startswith(pfx)]
            if kept and len(kept) < len(waits):
                si.on_wait = kept
                n += len(waits) - len(kept)
    return n


def _strip_dead_const_memsets(nc):
    """Delete framework const-AP memsets whose tensor is never read."""
    read = set()
    for blk in nc.m.functions[0].blocks:
        for ins in blk.instructions:
            for arg in getattr(ins, "ins", []) or []:
                ref = getattr(arg, "memref", None)
                if ref:
                    read.add(ref)
    removed = 0
    for blk in nc.m.functions[0].blocks:
        keep = []
        for ins in blk.instructions:
            if (type(ins).__name__ == "InstMemset"
                    and ins.sync_info is None
                    and getattr(ins.outs[0], "memref", "").startswith("const-")
                    and ins.outs[0].memref not in read):
                removed += 1
                continue
            keep.append(ins)
        if removed:
            del blk.instructions[:]
            for ins in keep:
                blk.instructions.append(ins)
    return removed


def _trim_tail_barrier(nc):
    """Drop the second all-engine barrier round at the kernel tail.

    The TileContext epilogue runs barrier -> semaphore reset -> barrier.
    The second barrier only fences engines against code that would run
    after the reset; this kernel's end block is the last block, so there
    is nothing to fence."""
    for blk in nc.m.functions[0].blocks:
        if not getattr(blk, "name", "").endswith("_end"):
            continue
        insts = list(blk.instructions)
        idx = None
        for i, ins in enumerate(insts):
            if (type(ins).__name__ == "InstISA"
                    and ins.engine == mybir.EngineType.Pool):
                idx = i
        if idx is None or idx + 1 >= len(insts):
            return 0
        tail = insts[idx + 1:]
        if any(type(t).__name__ not in ("InstDrain", "InstEventSemaphore")
               for t in tail):
            return 0
        del blk.instructions[:]
        for ins in insts[:idx + 1]:
            blk.instructions.append(ins)
        return len(tail)
    return 0


def _make_groups(slot_widths):
    """One slot per PSUM/ACT group. (Merging several narrow slots into
    one Exp activation with per-slot row-sums on the VectorEngine was
    measured 2.2x SLOWER than per-slot activations with fused accum_out
    — DVE tensor_reduce throughput does not keep up with ACT.)"""
    return [[s] for s in range(len(slot_widths))]


def _build_bass(slot_widths, repeats=1):
    n_slots = len(slot_widths)
    totc = int(np.sum(slot_widths))
    coffs = np.concatenate([[0], np.cumsum(slot_widths)]).astype(int)
    groups = _make_groups(slot_widths)
    nc = bacc.Bacc("TRN2", target_bir_lowering=False, debug=False,
                   num_devices=N_CORES)
    feats = nc.dram_tensor("feats", [64, n_slots * TILE_PTS], BF16,
                           kind="ExternalInput")
    coeffs = nc.dram_tensor("coeffs", [64, totc], BF16,
                            kind="ExternalInput")
    out = nc.dram_tensor("out", [128, n_slots], F32, kind="ExternalOutput")

    with tile.TileContext(nc) as tc:
        with (
            tc.tile_pool(name="const", bufs=1) as const_pool,
            tc.tile_pool(name="psum", bufs=2, space="PSUM") as psum_pool,
            tc.tile_pool(name="scratch", bufs=1) as scratch_pool,
            tc.tile_pool(name="acc", bufs=1) as acc_pool,
        ):
            FT = const_pool.tile([64, n_slots * TILE_PTS], BF16, tag="FT")
            nc.sync.dma_start(FT[:], feats.ap())
            CT = const_pool.tile([64, totc], BF16, tag="CT")
            nc.sync.dma_start(CT[:], coeffs.ap())
            accs = acc_pool.tile([128, n_slots], F32, tag="accs")
            sc0 = scratch_pool.tile([128, MAX_W], F32, tag="sc0")
            sc1 = scratch_pool.tile([128, MAX_W], F32, tag="sc1")

            for _r in range(repeats):
                for gi, grp in enumerate(groups):
                    ps = psum_pool.tile([128, MAX_W], F32, tag="ps")
                    slot_pos = []
                    po = 0
                    for s in grp:
                        w = slot_widths[s]
                        lhsT = FT[:, s * TILE_PTS:(s + 1) * TILE_PTS]
                        # chunk at PSUM bank boundaries: a matmul whose
                        # write crosses a 512-element bank edge is split
                        # by walrus into two ISA instructions, which
                        # breaks the Tile scheduler's semaphore counts
                        # (observed as a nondeterministic ACT/PE race)
                        o = 0
                        while o < w:
                            room = 512 - (po + o) % 512
                            cw = min(room, w - o)
                            nc.tensor.matmul(
                                ps[:, po + o:po + o + cw],
                                lhsT,
                                CT[:, coffs[s] + o:coffs[s] + o + cw],
                                start=True,
                                stop=True,
                            )
                            o += cw
                        slot_pos.append((s, po, w))
                        po += w
                    gw = po
                    sct = sc0 if gi % 2 == 0 else sc1
                    if len(grp) == 1:
                        s, _, w = slot_pos[0]
                        nc.scalar.activation(
                            sct[:, :w], ps[:, :w],
                            mybir.ActivationFunctionType.Exp,
                            accum_out=accs[:, s:s + 1],
                        )
                    else:
                        nc.scalar.activation(
                            sct[:, :gw], ps[:, :gw],
                            mybir.ActivationFunctionType.Exp,
                        )
                        for s, po_, w in slot_pos:
                            nc.vector.reduce_sum(
                                accs[:, s:s + 1],
                                sct[:, po_:po_ + w],
                                axis=mybir.AxisListType.X,
                            )
            nc.sync.dma_start(out.ap(), accs[:])
    _dedup_ldweights(nc)
    _strip_self_waits(nc)
    nc.compile()
    _trim_tail_barrier(nc)
    _strip_dead_const_memsets(nc)
    return nc


# ----------------------------------------------------------------- interface

def _assemble(results, slot_widths, units, order):
    """Per-core [128, n_slots] accumulators -> full [B] output in input
    order (summing partials of tiles split across units)."""
    tile_out = np.zeros((N_TILES, TILE_PTS), dtype=np.float64)
    for s in range(len(slot_widths)):
        for c in range(N_CORES):
            u = units[s][c]
            if u is None:
                continue
            t, _cols = u
            tile_out[t] += results[c]["out"][:, s].astype(np.float64)
    out_full = np.empty(B, dtype=np.float32)
    out_full[order] = tile_out.reshape(B).astype(np.float32)
    return out_full


def _run(inputs, trace=False):
    in_maps, slot_widths, units, order = _prepare(inputs)
    nc = _build_bass(slot_widths)
    res = run_bass_kernel_spmd(
        nc, in_maps, core_ids=list(range(N_CORES)), trace=trace
    )
    return _assemble(res.results, slot_widths, units, order), res


def kernel(x, means, cov_tril, weights):
    x = np.asarray(x)
    means = np.asarray(means)
    cov_tril = np.asarray(cov_tril)
    weights = np.asarray(weights)
    assert x.shape == (B, 3) and means.shape == (N, 3)
    assert cov_tril.shape == (N, 6) and weights.shape == (N,)
    out, _ = _run(
        {"x": x, "means": means, "cov_tril": cov_tril, "weights": weights}
    )
    return out


# revision 14
# speedup vs baseline: 1.2867x; 1.2867x over previous
"""Learnable 3D Gaussian field evaluation on 8 Trainium2 NeuronCores.

Reference computes, for B=32768 points x and N=4096 gaussians
(mean m_n, packed Cholesky cov_tril, weight w_n):

    out[b] = sum_n w_n * exp(-0.5 * (x_b - m_n)^T A_n (x_b - m_n)),
    A_n = (L_n L_n^T)^{-1}

Two key reformulations:

1. Quadratic-form matmul: the exponent is a quadratic in x, so with a
   10-dim feature vector f(x) = [x0^2, x1^2, x2^2, x0x1, x0x2, x1x2,
   x0, x1, x2, 1] (x centered) and per-gaussian coefficients c_n (with
   -0.5 and log w_n folded in), exponent[b, n] = f(x_b) . c_n — a
   TensorEngine matmul followed by exp + row-sum on the ScalarEngine
   (one activation instruction with accum_out). fp32 operands are each
   split into 3 bf16 components and the 6 significant cross products
   stacked along the contraction dim (K=60) -> ~fp32 precision.

2. Certified spatial culling: the exp work (B*N/8 elements per core on
   the only engine with an exp LUT) is the roofline. Points are
   Morton-sorted into 256 tiles of 128; for each tile, gaussians whose
   maximum possible contribution (upper bound via distance to the tile
   bbox and the largest covariance eigenvalue) is negligible are
   dropped. The drop budget is adaptive: sum of dropped upper bounds
   <= 1e-3 * (certified lower bound of the output anywhere in the
   tile), so the relative error is bounded by ~1e-3 per point by
   construction, for any input. Survivors (~12-15% here) are gathered
   on the host into dense per-tile coefficient slabs.

SPMD scheduling: all 8 cores share one instruction stream, so tile
work units (split at 2048 columns) are sorted by width and dealt
round-robin into groups of 8 — one slot per group, slot width = group
max. Per-core work is identical and balanced; each core's packed
operands carry its own unit's features/coefficients. Padding columns
encode exponent -30000 so they contribute exp(-30000) = 0.

Sharding: slots are data-parallel across 8 cores; no collectives.
"""

import sys

import numpy as np

try:
    import concourse.bass as bass  # noqa: F401
except ImportError:
    sys.path.insert(0, "/opt/trn_rl_repo")

import ml_dtypes

import concourse.bacc as bacc
import concourse.bass as bass  # noqa: F401
import concourse.mybir as mybir
import concourse.tile as tile
from concourse.bass_utils import run_bass_kernel_spmd

B, N = 32768, 4096
N_CORES = 8
TILE_PTS = 128                  # points per tile (PSUM partition dim)
N_TILES = B // TILE_PTS         # 256 spatial point-tiles
MAX_W = 2048                    # max slot width (half of PSUM, 4 banks)
PSUM_BUFS = 2                   # double-buffered PSUM: PE fills one while
                                # ACT drains the other (4-deep/1024-wide
                                # measured no better and is harder to time)
KSPLIT = 60                     # 6 bf16 cross products x 10 features
CENTER = 5.0
CULL_REL = 1e-3                 # culling error budget vs per-tile lower bound
PAD_EXP = -30000.0              # exponent encoded by padding columns

BF16 = mybir.dt.bfloat16
F32 = mybir.dt.float32


# ---------------------------------------------------------------- host math

def _build_coeffs(means, cov_tril, weights):
    """[N, 10] float64 coefficients c_n so that exponent = f(x') . c_n."""
    m = means.astype(np.float64) - CENTER
    ct = cov_tril.astype(np.float64)
    w = weights.astype(np.float64)
    eps = 1e-6
    L00 = np.exp(ct[:, 0]) + eps
    L11 = np.exp(ct[:, 2]) + eps
    L22 = np.exp(ct[:, 5]) + eps
    L10, L20, L21 = ct[:, 1], ct[:, 3], ct[:, 4]
    i00 = 1.0 / L00
    i11 = 1.0 / L11
    i22 = 1.0 / L22
    i10 = -L10 / (L00 * L11)
    i21 = -L21 / (L11 * L22)
    i20 = (L10 * L21 - L20 * L11) / (L00 * L11 * L22)
    A00 = i00 * i00 + i10 * i10 + i20 * i20
    A01 = i10 * i11 + i20 * i21
    A02 = i20 * i22
    A11 = i11 * i11 + i21 * i21
    A12 = i21 * i22
    A22 = i22 * i22
    Am0 = A00 * m[:, 0] + A01 * m[:, 1] + A02 * m[:, 2]
    Am1 = A01 * m[:, 0] + A11 * m[:, 1] + A12 * m[:, 2]
    Am2 = A02 * m[:, 0] + A12 * m[:, 1] + A22 * m[:, 2]
    mAm = m[:, 0] * Am0 + m[:, 1] * Am1 + m[:, 2] * Am2
    return np.stack(
        [
            -0.5 * A00, -0.5 * A11, -0.5 * A22,
            -A01, -A02, -A12,
            Am0, Am1, Am2,
            -0.5 * mAm + np.log(w),
        ],
        axis=1,
    )


def _build_feats(x):
    """[B, 10] float64 features of centered x."""
    xc = x.astype(np.float64) - CENTER
    x0, x1, x2 = xc[:, 0], xc[:, 1], xc[:, 2]
    return np.stack(
        [x0 * x0, x1 * x1, x2 * x2, x0 * x1, x0 * x2, x1 * x2,
         x0, x1, x2, np.ones_like(x0)],
        axis=1,
    )


def _split3_bf16(a64):
    """float64 -> three bf16 components with p0+p1+p2 covering ~24 bits."""
    p0 = a64.astype(ml_dtypes.bfloat16)
    r1 = a64 - p0.astype(np.float64)
    p1 = r1.astype(ml_dtypes.bfloat16)
    r2 = r1 - p1.astype(np.float64)
    p2 = r2.astype(ml_dtypes.bfloat16)
    return p0, p1, p2


def _split_stacks(f, c):
    """[60, B] and [60, N] bf16 stacks of the 6 significant cross products."""
    F0, F1, F2 = _split3_bf16(f)
    C0, C1, C2 = _split3_bf16(c)
    pairs = [(F0, C0), (F0, C1), (F1, C0), (F0, C2), (F1, C1), (F2, C0)]
    fstack = np.ascontiguousarray(
        np.concatenate([p[0].T for p in pairs], axis=0)
    ).astype(ml_dtypes.bfloat16)
    cstack = np.ascontiguousarray(
        np.concatenate([p[1].T for p in pairs], axis=0)
    ).astype(ml_dtypes.bfloat16)
    return fstack, cstack


def _kd_order(x):
    """Recursive median split (longest axis) into tiles of TILE_PTS points:
    compact bounding boxes everywhere, which is what the culling bound
    feeds on. Split sizes are kept multiples of TILE_PTS so every leaf is
    exactly one tile."""
    out = []

    def rec(ids):
        if len(ids) <= TILE_PTS:
            out.append(ids)
            return
        p = x[ids]
        ax = int(np.argmax(p.max(0) - p.min(0)))
        half = max(TILE_PTS, (len(ids) // 2 // TILE_PTS) * TILE_PTS)
        part = np.argpartition(p[:, ax], half)
        rec(ids[part[:half]])
        rec(ids[part[half:]])

    rec(np.arange(len(x)))
    return np.concatenate(out)


def _sigma_bounds(cov_tril):
    """Per-gaussian sqrt of smallest/largest eigenvalue of L L^T."""
    ct = cov_tril.astype(np.float64)
    eps = 1e-6
    L = np.zeros((N, 3, 3))
    L[:, 0, 0] = np.exp(ct[:, 0]) + eps
    L[:, 1, 1] = np.exp(ct[:, 2]) + eps
    L[:, 2, 2] = np.exp(ct[:, 5]) + eps
    L[:, 1, 0] = ct[:, 1]
    L[:, 2, 0] = ct[:, 3]
    L[:, 2, 1] = ct[:, 4]
    ev = np.linalg.eigvalsh(L @ L.transpose(0, 2, 1))
    return np.sqrt(np.maximum(ev[:, 0], 1e-30)), np.sqrt(ev[:, -1])


def _cull_tiles(xs, means, sig_min, sig_max, absw):
    """Per-tile certified culling of a tiled point set [T*TILE_PTS, 3].
    Drops gaussians with the smallest contribution upper bounds until the
    dropped total reaches CULL_REL times the certified output lower bound
    anywhere in the tile. Returns survivor index arrays per tile."""
    tiles = xs.reshape(-1, TILE_PTS, 3)
    lo = tiles.min(axis=1)
    hi = tiles.max(axis=1)
    surv = []
    for t in range(len(tiles)):
        dv = np.maximum(np.maximum(lo[t][None, :] - means,
                                   means - hi[t][None, :]), 0)
        dnear2 = (dv ** 2).sum(1)
        fv = np.maximum(np.abs(means - lo[t][None, :]),
                        np.abs(means - hi[t][None, :]))
        dfar2 = (fv ** 2).sum(1)
        ub = absw * np.exp(-0.5 * dnear2 / sig_max ** 2)
        out_lb = (absw * np.exp(-0.5 * dfar2 / sig_min ** 2)).sum()
        o = np.argsort(ub)
        ndrop = int(np.searchsorted(np.cumsum(ub[o]), CULL_REL * out_lb))
        surv.append(np.sort(o[ndrop:]).astype(np.int64))
    return surv


def _prepare(inputs):
    """Host prep: sort, cull, pack. Returns (in_maps, slot_widths, units,
    order) where units[s][c] = (tile, survivor_cols) for slot s, core c."""
    x = inputs["x"].astype(np.float64)
    means = inputs["means"].astype(np.float64)
    cov_tril = inputs["cov_tril"]
    weights = inputs["weights"].astype(np.float64)

    sig_min, sig_max = _sigma_bounds(cov_tril)
    absw = np.maximum(np.abs(weights), 1e-30)

    # pass 1: spatial sort + certified culling; find tiles blown up by
    # low-density outlier points (tiny certified lower bound -> huge
    # survivor count)
    order = _kd_order(x)
    surv = _cull_tiles(x[order], means, sig_min, sig_max, absw)
    fat = [t for t in range(len(surv)) if len(surv[t]) > 1024]
    cand = (np.concatenate([order[t * TILE_PTS:(t + 1) * TILE_PTS]
                            for t in fat])
            if fat else np.empty(0, np.int64))
    if len(cand) >= TILE_PTS:
        # exact per-point lower bound for candidate points; the TILE_PTS
        # worst become one dedicated tile so they stop inflating the
        # culling budget of whole spatial tiles
        d2 = ((x[cand][:, None, :] - means[None, :, :]) ** 2).sum(-1)
        lb_pt = (absw[None, :]
                 * np.exp(-0.5 * d2 / sig_min[None, :] ** 2)).sum(1)
        out_ids = cand[np.argsort(lb_pt)[:TILE_PTS]]
        rest = np.setdiff1d(np.arange(B), out_ids)
        order = np.concatenate([rest[_kd_order(x[rest])], out_ids])
        surv = _cull_tiles(x[order], means, sig_min, sig_max, absw)
    xs = x[order]

    # work units (tile, survivor column slice), split at MAX_W
    raw_units = []
    for t in range(N_TILES):
        cols = surv[t]
        for off in range(0, len(cols), MAX_W):
            raw_units.append((t, cols[off:off + MAX_W]))
    raw_units.sort(key=lambda u: -len(u[1]))
    n_slots = (len(raw_units) + N_CORES - 1) // N_CORES
    units = []          # units[s][c] -> (tile, cols) or None
    slot_widths = []
    for s in range(n_slots):
        grp = raw_units[s * N_CORES:(s + 1) * N_CORES]
        w = max(len(u[1]) for u in grp)
        w = ((w + 127) // 128) * 128
        slot_widths.append(w)
        grp = grp + [None] * (N_CORES - len(grp))
        units.append(grp)

    # packed per-core operands
    feats64 = _build_feats(xs)
    coeffs64 = _build_coeffs(means, cov_tril, weights)
    fstack, cstack = _split_stacks(feats64, coeffs64)      # [60,B], [60,N]
    totc = int(np.sum(slot_widths))
    pad_col = np.zeros((64,), dtype=ml_dtypes.bfloat16)
    pad_col[9] = PAD_EXP       # block-0 constant-feature row -> exp -> 0

    in_maps = []
    for c in range(N_CORES):
        ft = np.zeros((64, n_slots * TILE_PTS), dtype=ml_dtypes.bfloat16)
        cf = np.tile(pad_col[:, None], (1, totc))
        coff = 0
        for s in range(n_slots):
            u = units[s][c]
            if u is not None:
                t, cols = u
                ft[:KSPLIT, s * TILE_PTS:(s + 1) * TILE_PTS] = \
                    fstack[:, t * TILE_PTS:(t + 1) * TILE_PTS]
                cf[:KSPLIT, coff:coff + len(cols)] = cstack[:, cols]
                cf[KSPLIT:, coff:coff + len(cols)] = 0.0
            coff += slot_widths[s]
        in_maps.append({"feats": np.ascontiguousarray(ft),
                        "coeffs": np.ascontiguousarray(cf)})
    return in_maps, slot_widths, units, order


# ------------------------------------------------------------- device kernel

def _dedup_ldweights(nc):
    """Remove redundant InstLdweights: consecutive matmuls reusing the same
    stationary operand only need the first load. Only drops loads that carry
    no semaphore waits/updates and whose weights AP matches the previous
    load, with nothing but matmuls in between on the PE stream."""
    removed = 0
    for blk in nc.m.functions[0].blocks:
        keep = []
        last_sig = None
        for ins in blk.instructions:
            if getattr(ins, "engine", None) == mybir.EngineType.PE:
                tname = type(ins).__name__
                if tname == "InstLdweights":
                    sig = repr(ins.ins[0])
                    if sig == last_sig and ins.sync_info is None:
                        removed += 1
                        continue
                    last_sig = sig
                elif tname != "InstMatmult":
                    last_sig = None
            keep.append(ins)
        if removed:
            del blk.instructions[:]
            for ins in keep:
                blk.instructions.append(ins)
    return removed


_ENGINE_SEM_PREFIX = {
    mybir.EngineType.PE: "PE_",
    mybir.EngineType.Activation: "Activation_",
}


def _strip_self_waits(nc):
    """Drop same-engine semaphore waits from multi-wait PE/ACT instructions.

    Engines execute their instruction streams in order, so a wait on the
    engine's own completion semaphore is redundant whenever the instruction
    also carries the cross-engine wait that actually orders it."""
    n = 0
    for blk in nc.m.functions[0].blocks:
        for ins in blk.instructions:
            pfx = _ENGINE_SEM_PREFIX.get(getattr(ins, "engine", None))
            si = ins.sync_info
            if pfx is None or si is None or not si.on_wait:
                continue
            waits = list(si.on_wait)
            if len(waits) < 2:
                continue
            kept = [w for w in waits if not w.ant_name.startswith(pfx)]
            if kept and len(kept) < len(waits):
                si.on_wait = kept
                n += len(waits) - len(kept)
    return n


def _strip_dead_const_memsets(nc):
    """Delete framework const-AP memsets whose tensor is never read."""
    read = set()
    for blk in nc.m.functions[0].blocks:
        for ins in blk.instructions:
            for arg in getattr(ins, "ins", []) or []:
                ref = getattr(arg, "memref", None)
                if ref:
                    read.add(ref)
    removed = 0
    for blk in nc.m.functions[0].blocks:
        keep = []
        for ins in blk.instructions:
            if (type(ins).__name__ == "InstMemset"
                    and ins.sync_info is None
                    and getattr(ins.outs[0], "memref", "").startswith("const-")
                    and ins.outs[0].memref not in read):
                removed += 1
                continue
            keep.append(ins)
        if removed:
            del blk.instructions[:]
            for ins in keep:
                blk.instructions.append(ins)
    return removed


def _trim_tail_barrier(nc):
    """Drop the second all-engine barrier round at the kernel tail.

    The TileContext epilogue runs barrier -> semaphore reset -> barrier.
    The second barrier only fences engines against code that would run
    after the reset; this kernel's end block is the last block, so there
    is nothing to fence."""
    for blk in nc.m.functions[0].blocks:
        if not getattr(blk, "name", "").endswith("_end"):
            continue
        insts = list(blk.instructions)
        idx = None
        for i, ins in enumerate(insts):
            if (type(ins).__name__ == "InstISA"
                    and ins.engine == mybir.EngineType.Pool):
                idx = i
        if idx is None or idx + 1 >= len(insts):
            return 0
        tail = insts[idx + 1:]
        if any(type(t).__name__ not in ("InstDrain", "InstEventSemaphore")
               for t in tail):
            return 0
        del blk.instructions[:]
        for ins in insts[:idx + 1]:
            blk.instructions.append(ins)
        return len(tail)
    return 0


def _make_groups(slot_widths):
    """One slot per PSUM/ACT group. (Merging several narrow slots into
    one Exp activation with per-slot row-sums on the VectorEngine was
    measured 2.2x SLOWER than per-slot activations with fused accum_out
    — DVE tensor_reduce throughput does not keep up with ACT.)"""
    return [[s] for s in range(len(slot_widths))]


def _build_bass(slot_widths, repeats=1):
    n_slots = len(slot_widths)
    totc = int(np.sum(slot_widths))
    coffs = np.concatenate([[0], np.cumsum(slot_widths)]).astype(int)
    groups = _make_groups(slot_widths)
    nc = bacc.Bacc("TRN2", target_bir_lowering=False, debug=False,
                   num_devices=N_CORES)
    feats = nc.dram_tensor("feats", [64, n_slots * TILE_PTS], BF16,
                           kind="ExternalInput")
    coeffs = nc.dram_tensor("coeffs", [64, totc], BF16,
                            kind="ExternalInput")
    out = nc.dram_tensor("out", [128, n_slots], F32, kind="ExternalOutput")

    with tile.TileContext(nc) as tc:
        with (
            tc.tile_pool(name="const", bufs=1) as const_pool,
            tc.tile_pool(name="psum", bufs=PSUM_BUFS, space="PSUM") as psum_pool,
            tc.tile_pool(name="scratch", bufs=1) as scratch_pool,
            tc.tile_pool(name="acc", bufs=1) as acc_pool,
        ):
            FT = const_pool.tile([64, n_slots * TILE_PTS], BF16, tag="FT")
            nc.sync.dma_start(FT[:], feats.ap())
            CT = const_pool.tile([64, totc], BF16, tag="CT")
            nc.sync.dma_start(CT[:], coeffs.ap())
            accs = acc_pool.tile([128, n_slots], F32, tag="accs")
            sc0 = scratch_pool.tile([128, MAX_W], F32, tag="sc0")
            sc1 = scratch_pool.tile([128, MAX_W], F32, tag="sc1")

            for _r in range(repeats):
                for gi, grp in enumerate(groups):
                    ps = psum_pool.tile([128, MAX_W], F32, tag="ps")
                    slot_pos = []
                    po = 0
                    for s in grp:
                        w = slot_widths[s]
                        lhsT = FT[:, s * TILE_PTS:(s + 1) * TILE_PTS]
                        # chunk at PSUM bank boundaries: a matmul whose
                        # write crosses a 512-element bank edge is split
                        # by walrus into two ISA instructions, which
                        # breaks the Tile scheduler's semaphore counts
                        # (observed as a nondeterministic ACT/PE race)
                        o = 0
                        while o < w:
                            room = 512 - (po + o) % 512
                            cw = min(room, w - o)
                            nc.tensor.matmul(
                                ps[:, po + o:po + o + cw],
                                lhsT,
                                CT[:, coffs[s] + o:coffs[s] + o + cw],
                                start=True,
                                stop=True,
                            )
                            o += cw
                        slot_pos.append((s, po, w))
                        po += w
                    gw = po
                    sct = sc0 if gi % 2 == 0 else sc1
                    if len(grp) == 1:
                        s, _, w = slot_pos[0]
                        nc.scalar.activation(
                            sct[:, :w], ps[:, :w],
                            mybir.ActivationFunctionType.Exp,
                            accum_out=accs[:, s:s + 1],
                        )
                    else:
                        nc.scalar.activation(
                            sct[:, :gw], ps[:, :gw],
                            mybir.ActivationFunctionType.Exp,
                        )
                        for s, po_, w in slot_pos:
                            nc.vector.reduce_sum(
                                accs[:, s:s + 1],
                                sct[:, po_:po_ + w],
                                axis=mybir.AxisListType.X,
                            )
            nc.sync.dma_start(out.ap(), accs[:])
    _dedup_ldweights(nc)
    _strip_self_waits(nc)
    nc.compile()
    _trim_tail_barrier(nc)
    _strip_dead_const_memsets(nc)
    return nc


# ----------------------------------------------------------------- interface

def _assemble(results, slot_widths, units, order):
    """Per-core [128, n_slots] accumulators -> full [B] output in input
    order (summing partials of tiles split across units)."""
    tile_out = np.zeros((N_TILES, TILE_PTS), dtype=np.float64)
    for s in range(len(slot_widths)):
        for c in range(N_CORES):
            u = units[s][c]
            if u is None:
                continue
            t, _cols = u
            tile_out[t] += results[c]["out"][:, s].astype(np.float64)
    out_full = np.empty(B, dtype=np.float32)
    out_full[order] = tile_out.reshape(B).astype(np.float32)
    return out_full


def _run(inputs, trace=False):
    in_maps, slot_widths, units, order = _prepare(inputs)
    nc = _build_bass(slot_widths)
    res = run_bass_kernel_spmd(
        nc, in_maps, core_ids=list(range(N_CORES)), trace=trace
    )
    return _assemble(res.results, slot_widths, units, order), res


def kernel(x, means, cov_tril, weights):
    x = np.asarray(x)
    means = np.asarray(means)
    cov_tril = np.asarray(cov_tril)
    weights = np.asarray(weights)
    assert x.shape == (B, 3) and means.shape == (N, 3)
    assert cov_tril.shape == (N, 6) and weights.shape == (N,)
    out, _ = _run(
        {"x": x, "means": means, "cov_tril": cov_tril, "weights": weights}
    )
    return out
